# revision 15
# baseline (speedup 1.0000x reference)
"""Trainium2 Bass kernel for ChemicalNet (per-species MLP / MoE routing).

Strategy
--------
Only atoms whose species is in {1, 6, 7, 8} produce output (others are 0),
and each such atom only needs ITS OWN species' 3-layer MLP.  The reference
runs all 4 expert networks on all atoms; we route on the host instead:

- host: map species -> expert index, collect per-expert atom index lists
- shard: 2 cores per expert, each core gets half of that expert's atoms
  (the per-core in_map carries that expert's weights, so the single SPMD
  program is expert-agnostic)
- host passes the gathered embedding columns TRANSPOSED ([128, n]) so the
  device needs no transposes: PE contracts over the partition axis directly
- device: L1 matmul+SiLU, L2 matmul (2-step K accum)+SiLU, L3 matmul -> [1,n]
- host scatters the compact per-core outputs back to the full [N, 1] output

Performance notes (from NTFF traces of the fp32r version)
---------------------------------------------------------
- fp32r matmuls run fp32_mode=HIGH: ~2 cycles/col and no fast-weight-load.
  bf16 runs 1 col/cycle with FWL (4x faster LDWEIGHTS) and halves the
  embedding DMA.  Host-simulated bf16 end-to-end error is 4e-3 (threshold
  2e-2), so bf16 is the default.
- The PE HAM clock gate keeps the array at 1.2 GHz until it has been busy
  ~3.4us.  A burst of dummy warm-up matmuls at t=0 (while input DMAs run)
  moves the 2.4 GHz transition before the first real matmul.
- Input DMAs previously serialized ~5us on the sync HWDGE queue; emb chunks
  now alternate between the sync and scalar HWDGE queues.
- The scalar engine's ACTIVATE stream (SiLU) is the steady-state bottleneck
  (~1 elem/cycle/lane @ 1.2 GHz, dtype independent).

Per-chunk (512 atoms) the two 128-row halves of the hidden layer land in one
[128, 1024] PSUM tile so a single ACTIVATE applies SiLU to both.  That merge
needs a bias constant along the free axis; biases in this problem are
identically zero, which the host verifies -- nonzero-bias inputs take a
(slower) per-half ACTIVATE path with per-partition bias.

The layer-3 [1, F] matmul accumulates into a corner of the layer-2 PSUM
tile after its ACTIVATE has read it (WAR handled by Tile), so all 8 PSUM
banks go to the 4-deep [128, 1024] pipeline pool.

All shapes are compile-time constants derived from the actual input
(the Bass program is built fresh per call).
"""

import numpy as np

import concourse.bass as bass
import concourse.tile as tile
from concourse import bacc, mybir
from concourse.bass_utils import run_bass_kernel_spmd

N_CORES = 8
NSPECIES = 4
SPECIES_Z = np.array([1, 6, 7, 8], dtype=np.int32)
MAXIDX = 118
D = 128          # embedding dim
H = 256          # hidden dim
F = 512          # atom-chunk size (one PSUM bank of fp32)
FP = mybir.dt.float32
SILU = mybir.ActivationFunctionType.Silu
WARM_MMS = 13    # dummy matmuls at t=0 to trip the HAM clock gate early
WCOLS = 3 * H + 2  # combined weights tile: w1 [*,0:256], w2 [*,256:768], w3 [*,768:770]


def _build_program(npad: int, zero_bias: bool, mmdt):
    """One SPMD program: a 3-layer per-expert MLP over `npad` atom columns.

    Structure: the atom axis is processed in GROUPS of up to 1024 columns
    (a small first group so the first ACTIVATE starts early).  PSUM is
    split into two fixed [128, 2048] regions -- R1 holds L1 pre-activations
    (m-half 0 | m-half 1 along the free axis), R2 holds L2's plus, after
    ACT2 has read it, the [1, f] L3 accumulation in partition row 0.  Big
    groups mean few, large ACTIVATEs (the scalar engine is the bottleneck:
    1 elem/cycle/lane + ~352 cycles tax per instruction).  The ACTIVATE
    queue is software-pipelined (A1(g+1) is enqueued before A2(g)) so the
    in-order scalar queue never head-of-line blocks.
    """
    nc = bacc.Bacc("TRN2", target_bir_lowering=False, debug=False,
                   num_devices=N_CORES)

    embT_d = nc.dram_tensor("embT", [D, npad], mmdt, kind="ExternalInput")
    w_d = nc.dram_tensor("w", [D, WCOLS], mmdt, kind="ExternalInput")
    if not zero_bias:
        b1_d = nc.dram_tensor("b1", [128, 2], FP, kind="ExternalInput")
        b2_d = nc.dram_tensor("b2", [128, 2], FP, kind="ExternalInput")
        b3_d = nc.dram_tensor("b3", [1, 1], FP, kind="ExternalInput")
    out_d = nc.dram_tensor("out", [1, npad], FP, kind="ExternalOutput")

    # groups over the atom axis: [384, 1024, ..., remainder]
    groups = []
    c = 0
    g0 = min(384, npad)
    groups.append((0, g0))
    c = g0
    while npad - c > 1024:
        groups.append((c, 1024))
        c += 1024
    if npad - c:
        groups.append((c, npad - c))
    G = len(groups)

    def goff(f):
        # m-half-1 offset inside a PSUM region: 512-aligned so every
        # matmul output stays inside one PSUM bank
        return -(-f // 512) * 512

    def subs(f):
        return [(j * 512, min(512, f - j * 512)) for j in range(-(-f // 512))]

    # Input DMA slabs.  Descriptor generation is the DMA bottleneck
    # (~90ns/descriptor/engine, 1 descriptor per partition per transfer,
    # serial per HWDGE ring), so: 4 slabs, split across the two rings by
    # arrival deadline.  scalar ring: slab0 (gates group 0), s1b, s2b;
    # sync ring: weights (gates everything), s1a, s2a, then output DMAs.
    bounds = [0, groups[0][1]]
    if G > 1:
        g1end = groups[1][0] + groups[1][1]
        mid = (groups[0][1] + g1end) // 2 // 2 * 2
        bounds += [mid, g1end]
        if g1end < npad:
            mid2 = (g1end + npad) // 2 // 2 * 2
            bounds += [mid2, npad]
    bounds = sorted(set(b for b in bounds if b <= npad))
    slabs = [(a, b - a) for a, b in zip(bounds, bounds[1:]) if b > a]

    with tile.TileContext(nc) as tc:
        with (
            tc.tile_pool(name="singles", bufs=1) as singles,
            tc.tile_pool(name="ps1", bufs=1, space="PSUM") as psp1,
            tc.tile_pool(name="ps2", bufs=1, space="PSUM") as psp2,
        ):
            R1 = psp1.tile([128, 2048], FP)
            R2 = psp2.tile([128, 2048], FP)

            # --- t=0: warm the PE (HAM clock gate) with dummy matmuls on a
            # zeroed tile while the input DMAs stream in.  Disjoint 128-col
            # PSUM slices keep them independent (no WAW chain).
            warm_w = singles.tile([128, 128], mmdt)
            nc.vector.memset(warm_w[:], 0.0)
            for r in range(WARM_MMS):
                j = r % 8
                nc.tensor.matmul(R2[:, j * 128:(j + 1) * 128],
                                 warm_w[:], warm_w[:], start=True, stop=True)

            # preload the SiLU table set while input DMAs run
            warm_act = singles.tile([128, 1], FP)
            nc.vector.memset(warm_act[:], 0.0)
            nc.scalar.activation(warm_act[:], warm_act[:], SILU)

            emb_t = singles.tile([D, npad], mmdt)
            out_t = singles.tile([1, npad], FP)
            w_t = singles.tile([D, WCOLS], mmdt)

            nc.scalar.dma_start(emb_t[:, 0:slabs[0][1]],
                                embT_d[:, 0:slabs[0][1]])
            nc.sync.dma_start(w_t[:], w_d[:])
            for si, (s0, sw) in enumerate(slabs[1:], start=1):
                eng = nc.sync if si % 2 == 1 else nc.scalar
                eng.dma_start(emb_t[:, s0:s0 + sw], embT_d[:, s0:s0 + sw])

            if not zero_bias:
                b1_t = singles.tile([128, 2], FP)
                nc.gpsimd.dma_start(b1_t[:], b1_d[:])
                b2_t = singles.tile([128, 2], FP)
                nc.gpsimd.dma_start(b2_t[:], b2_d[:])
                b3_t = singles.tile([1, 1], FP)
                nc.gpsimd.dma_start(b3_t[:], b3_d[:])

            def act(z_t, ps_t, f, b_t):
                """SiLU both m-halves of a psum region -> z SBUF (one
                instruction over [0, goff+f) -- the [f, goff) gap columns
                are unused garbage, activating them is harmless)."""
                off = goff(f)
                if zero_bias:
                    nc.scalar.activation(z_t[:, :off + f], ps_t[:, :off + f],
                                         SILU)
                else:
                    for m in range(2):
                        nc.scalar.activation(
                            z_t[:, m * off:m * off + f],
                            ps_t[:, m * off:m * off + f], SILU,
                            bias=b_t[:, m:m + 1])

            z1s, z2s = {}, {}

            def emit_l1(g):
                c0, f = groups[g]
                off = goff(f)
                for m in range(2):
                    for so, sw in subs(f):
                        nc.tensor.matmul(
                            R1[:, m * off + so:m * off + so + sw],
                            w_t[:, m * 128:(m + 1) * 128],
                            emb_t[:, c0 + so:c0 + so + sw],
                            start=True, stop=True)
                z1 = singles.tile([128, off + f], mmdt, name=f"z1_{g}")
                act(z1, R1, f, None if zero_bias else b1_t)
                z1s[g] = z1

            def emit_l2(g):
                c0, f = groups[g]
                z1 = z1s[g]
                off = goff(f)
                # m-half 1 first, and m-half 0's low-bank sub last: the
                # DVE copy of the previous group's L3 row still reads
                # R2[0:1, 0:f_prev], so the overlapping writes go last.
                for m in (1, 0):
                    for so, sw in reversed(subs(f)) if m == 0 else subs(f):
                        for k in range(2):
                            nc.tensor.matmul(
                                R2[:, m * off + so:m * off + so + sw],
                                w_t[:, (1 + k) * H + m * 128:
                                    (1 + k) * H + m * 128 + 128],
                                z1[:, k * off + so:k * off + so + sw],
                                start=(k == 0), stop=(k == 1))
                z2 = singles.tile([128, off + f], mmdt, name=f"z2_{g}")
                act(z2, R2, f, None if zero_bias else b2_t)
                z2s[g] = z2

            def emit_l3(g):
                c0, f = groups[g]
                z2 = z2s[g]
                off = goff(f)
                # L3 accumulates into partition row 0 of R2 after ACT2's
                # read (WAR handled by Tile); the next group's L2 m-half-0
                # overwrites it only after the DVE copy below has read it.
                for so, sw in subs(f):
                    ps3 = R2[0:1, so:so + sw]
                    nc.tensor.matmul(ps3, w_t[:, 3 * H:3 * H + 1],
                                     z2[:, so:so + sw],
                                     start=True, stop=False)
                    nc.tensor.matmul(ps3, w_t[:, 3 * H + 1:3 * H + 2],
                                     z2[:, off + so:off + so + sw],
                                     start=False, stop=True)
                if zero_bias:
                    nc.vector.tensor_copy(out_t[:, c0:c0 + f], R2[0:1, 0:f])
                else:
                    nc.vector.tensor_scalar_add(out_t[:, c0:c0 + f],
                                                R2[0:1, 0:f], b3_t[0:1, 0:1])

            # software-pipelined emission; ACT queue order is
            # A1(0), A1(1), A2(0), A1(2), A2(1), ..., A2(G-1)
            emit_l1(0)
            if G > 1:
                emit_l1(1)
            emit_l2(0)
            for g in range(1, G):
                if g + 1 < G:
                    emit_l1(g + 1)
                emit_l3(g - 1)
                if g == G - 1 and G > 1:
                    bnd = groups[g][0]
                    nc.sync.dma_start(out_d[:, :bnd], out_t[:, :bnd])
                emit_l2(g)
            emit_l3(G - 1)
            bnd = groups[G - 1][0] if G > 1 else 0
            nc.sync.dma_start(out_d[:, bnd:npad], out_t[:, bnd:npad])

    nc.compile()
    return nc


def _route(species: np.ndarray):
    """species values -> expert idx (-1 unknown); per-core row assignments."""
    conv = np.full(MAXIDX + 2, -1, dtype=np.int32)
    conv[SPECIES_Z] = np.arange(NSPECIES, dtype=np.int32)
    idx = conv[species]
    core_rows = []
    for s in range(NSPECIES):
        rows = np.flatnonzero(idx == s)
        h = (len(rows) + 1) // 2
        core_rows.append(rows[:h])
        core_rows.append(rows[h:])
    return core_rows


def _run(inputs: dict, trace: bool = False, dtype_mode: str = "bf16"):
    species = inputs["species"]
    embedding = np.ascontiguousarray(inputs["embedding"], dtype=np.float32)
    n_atoms = species.shape[0]
    out_full = np.zeros((n_atoms, 1), dtype=np.float32)

    core_rows = _route(np.asarray(species))
    nmax = max(len(r) for r in core_rows)
    if nmax == 0:
        return out_full, None
    npad = -(-nmax // 8) * 8

    zero_bias = all(
        not np.any(np.asarray(inputs[k])) for k in ("b1", "b2", "b3"))
    mmdt = {"bf16": mybir.dt.bfloat16,
            "f32r": mybir.dt.float32r,
            "fp32": FP}[dtype_mode]
    np_mm = mybir.dt.np(mmdt)
    nc = _build_program(npad, zero_bias, mmdt)

    in_maps = []
    for c in range(N_CORES):
        s = c // 2
        rows = core_rows[c]
        embT = np.zeros((D, npad), dtype=np_mm)
        if len(rows):
            embT[:, :len(rows)] = embedding[rows].T.astype(np_mm)
        # combined weights tile: [w1 | w2 row-half 0 | w2 row-half 1 | w3]
        w = np.zeros((D, WCOLS), dtype=np_mm)
        w[:, 0:H] = np.asarray(inputs["W1"][s], dtype=np.float32).astype(np_mm)
        w2 = np.asarray(inputs["W2"][s], dtype=np.float32).astype(np_mm)
        w[:, H:2 * H] = w2[0:128, :]
        w[:, 2 * H:3 * H] = w2[128:256, :]
        w[:, 3 * H:3 * H + 2] = np.asarray(
            inputs["W3"][s], dtype=np.float32).reshape(2, 128).T.astype(np_mm)
        im = {
            "embT": embT,
            "w": np.ascontiguousarray(w),
        }
        if not zero_bias:
            im["b1"] = np.ascontiguousarray(
                np.asarray(inputs["b1"][s], dtype=np.float32).reshape(2, 128).T)
            im["b2"] = np.ascontiguousarray(
                np.asarray(inputs["b2"][s], dtype=np.float32).reshape(2, 128).T)
            im["b3"] = np.asarray(inputs["b3"][s], dtype=np.float32).reshape(1, 1)
        in_maps.append(im)

    res = run_bass_kernel_spmd(nc, in_maps, core_ids=list(range(N_CORES)),
                               trace=trace)
    for c in range(N_CORES):
        rows = core_rows[c]
        if len(rows):
            out_full[rows, 0] = res.results[c]["out"][0, :len(rows)]
    return out_full, res


def kernel(**inputs) -> np.ndarray:
    out, _ = _run(inputs, trace=False)
    return out


# revision 16
# speedup vs baseline: 1.3679x; 1.3679x over previous
"""Trainium2 Bass kernel for ChemicalNet (per-species MLP / MoE routing).

Strategy
--------
Only atoms whose species is in {1, 6, 7, 8} produce output (others are 0),
and each such atom only needs ITS OWN species' 3-layer MLP.  The reference
runs all 4 expert networks on all atoms; we route on the host instead:

- host: map species -> expert index, collect per-expert atom index lists
- shard: 2 cores per expert, each core gets half of that expert's atoms
  (the per-core in_map carries that expert's weights, so the single SPMD
  program is expert-agnostic)
- host passes the gathered embedding columns TRANSPOSED ([128, n]) so the
  device needs no transposes: PE contracts over the partition axis directly
- device: L1 matmul+SiLU, L2 matmul (2-step K accum)+SiLU, L3 matmul -> [1,n]
- host scatters the compact per-core outputs back to the full [N, 1] output

Performance notes (from NTFF traces of the fp32r version)
---------------------------------------------------------
- fp32r matmuls run fp32_mode=HIGH: ~2 cycles/col and no fast-weight-load.
  bf16 runs 1 col/cycle with FWL (4x faster LDWEIGHTS) and halves the
  embedding DMA.  Host-simulated bf16 end-to-end error is 4e-3 (threshold
  2e-2), so bf16 is the default.
- The PE HAM clock gate keeps the array at 1.2 GHz until it has been busy
  ~3.4us.  A burst of dummy warm-up matmuls at t=0 (while input DMAs run)
  moves the 2.4 GHz transition before the first real matmul.
- Input DMAs previously serialized ~5us on the sync HWDGE queue; emb chunks
  now alternate between the sync and scalar HWDGE queues.
- The scalar engine's ACTIVATE stream (SiLU) is the steady-state bottleneck
  (~1 elem/cycle/lane @ 1.2 GHz, dtype independent).

Per-chunk (512 atoms) the two 128-row halves of the hidden layer land in one
[128, 1024] PSUM tile so a single ACTIVATE applies SiLU to both.  That merge
needs a bias constant along the free axis; biases in this problem are
identically zero, which the host verifies -- nonzero-bias inputs take a
(slower) per-half ACTIVATE path with per-partition bias.

The layer-3 [1, F] matmul accumulates into a corner of the layer-2 PSUM
tile after its ACTIVATE has read it (WAR handled by Tile), so all 8 PSUM
banks go to the 4-deep [128, 1024] pipeline pool.

All shapes are compile-time constants derived from the actual input
(the Bass program is built fresh per call).
"""

import numpy as np

import concourse.bass as bass
import concourse.tile as tile
from concourse import bacc, mybir
from concourse.bass_utils import run_bass_kernel_spmd

N_CORES = 8
NSPECIES = 4
SPECIES_Z = np.array([1, 6, 7, 8], dtype=np.int32)
MAXIDX = 118
D = 128          # embedding dim
H = 256          # hidden dim
F = 512          # atom-chunk size (one PSUM bank of fp32)
FP = mybir.dt.float32
SILU = mybir.ActivationFunctionType.Silu
WARM_MMS = 15    # dummy matmuls at t=0 to trip the HAM clock gate early
WCOLS = 3 * H + 2  # combined weights tile: w1 [*,0:256], w2 [*,256:768], w3 [*,768:770]


def _build_program(npad: int, zero_bias: bool, mmdt):
    """One SPMD program: a 3-layer per-expert MLP over `npad` atom columns.

    F=512 atom chunks ride a 4-deep rotation of [128, 1024] PSUM tiles
    (m-half 0 | m-half 1 along the free axis), which is the finest pipeline
    grain 8 PSUM banks allow: the PE runs L1 of chunk c+2 while the scalar
    engine (the bottleneck: SiLU at 1 elem/cycle/lane + ~352 cycles/instr
    tax) drains chunk c.  The ACTIVATE queue is software-pipelined
    (A1(c+2) sits between A2(c) and A2(c+1)) so the in-order scalar queue
    never head-of-line blocks.
    """
    nc = bacc.Bacc("TRN2", target_bir_lowering=False, debug=False,
                   num_devices=N_CORES)

    embT_d = nc.dram_tensor("embT", [D, npad], mmdt, kind="ExternalInput")
    w_d = nc.dram_tensor("w", [D, WCOLS], mmdt, kind="ExternalInput")
    if not zero_bias:
        b1_d = nc.dram_tensor("b1", [128, 2], FP, kind="ExternalInput")
        b2_d = nc.dram_tensor("b2", [128, 2], FP, kind="ExternalInput")
        b3_d = nc.dram_tensor("b3", [1, 1], FP, kind="ExternalInput")
    out_d = nc.dram_tensor("out", [1, npad], FP, kind="ExternalOutput")

    # ramped chunk sizes: small first chunks let the first ACTIVATEs start
    # while the bulk of the embedding is still streaming in
    sizes = []
    for s in (128, 256):
        if sum(sizes) + s <= npad:
            sizes.append(s)
    while npad - sum(sizes) > F:
        sizes.append(F)
    if npad - sum(sizes):
        sizes.append(npad - sum(sizes))
    chunks = []
    c0 = 0
    for s in sizes:
        chunks.append((c0, s))
        c0 += s
    nch = len(chunks)

    # Input DMA slabs.  Descriptor generation is the DMA bottleneck
    # (~90ns/descriptor/engine, 1 descriptor per partition per transfer,
    # serial per HWDGE ring), so the embedding streams in as a few wide
    # slabs, and the two transfers gating chunk 0 are tiny and lead their
    # rings: scalar ring [0:128], [128:384], s2; sync ring: weights, s1.
    bounds = [0]
    for b in (chunks[0][1], 384):
        if bounds[-1] < b <= npad:
            bounds.append(b)
    rest = npad - bounds[-1]
    if rest > 0:
        mid = bounds[-1] + (rest + 3) // 4 * 2
        if mid < npad:
            bounds.append(mid)
        bounds.append(npad)
    slabs = [(a, b - a) for a, b in zip(bounds, bounds[1:])]
    # ring assignment: slab0 (+384-slab) scalar, s1 sync, s2 scalar
    scalar_slabs = slabs[:2] + slabs[3:4]
    sync_slabs = slabs[2:3]

    with tile.TileContext(nc) as tc:
        with (
            tc.tile_pool(name="singles", bufs=1) as singles,
            tc.tile_pool(name="z1p", bufs=nch) as z1p,
            tc.tile_pool(name="z2p", bufs=nch) as z2p,
            tc.tile_pool(name="ps", bufs=4, space="PSUM") as psp,
        ):
            # --- t=0: warm the PE (HAM clock gate) with dummy matmuls on a
            # zeroed tile while the input DMAs stream in.  Disjoint 128-col
            # PSUM slices keep them independent (no WAW chain).
            warm_w = singles.tile([128, 128], mmdt)
            nc.vector.memset(warm_w[:], 0.0)
            warm_ps = psp.tile([128, 2 * F], FP, tag="ps", name="warm_ps")
            for r in range(WARM_MMS):
                j = r % 8
                nc.tensor.matmul(warm_ps[:, j * 128:(j + 1) * 128],
                                 warm_w[:], warm_w[:], start=True, stop=True)

            emb_t = singles.tile([D, npad], mmdt)
            out_t = singles.tile([1, npad], FP)
            w_t = singles.tile([D, WCOLS], mmdt)

            for s0, sw in scalar_slabs:
                nc.scalar.dma_start(emb_t[:, s0:s0 + sw],
                                    embT_d[:, s0:s0 + sw])
            nc.sync.dma_start(w_t[:], w_d[:])
            for s0, sw in sync_slabs:
                nc.sync.dma_start(emb_t[:, s0:s0 + sw], embT_d[:, s0:s0 + sw])

            # preload the SiLU table set while input DMAs run (emitted after
            # the dma_starts so the scalar HWDGE ring leads with slab 0)
            warm_act = singles.tile([128, 1], FP)
            nc.vector.memset(warm_act[:], 0.0)
            nc.scalar.activation(warm_act[:], warm_act[:], SILU)

            if not zero_bias:
                b1_t = singles.tile([128, 2], FP)
                nc.gpsimd.dma_start(b1_t[:], b1_d[:])
                b2_t = singles.tile([128, 2], FP)
                nc.gpsimd.dma_start(b2_t[:], b2_d[:])
                b3_t = singles.tile([1, 1], FP)
                nc.gpsimd.dma_start(b3_t[:], b3_d[:])

            def m_off(f):
                # matmul output must stay inside one 512-col PSUM bank:
                # pack the m1 half right after m0 only when both fit bank 0
                return f if 2 * f <= F else F

            def act_pair(z_t, ps_t, f, b_t):
                """SiLU both m-halves of a psum tile -> z SBUF (one
                instruction; for off==F > f the unused gap columns are
                activated too, which is harmless and cheaper)."""
                off = m_off(f)
                if zero_bias:
                    nc.scalar.activation(z_t[:, :off + f], ps_t[:, :off + f],
                                         SILU)
                else:
                    for m in range(2):
                        nc.scalar.activation(
                            z_t[:, m * off:m * off + f],
                            ps_t[:, m * off:m * off + f], SILU,
                            bias=b_t[:, m:m + 1])

            z1s, z2s, ps2s = {}, {}, {}

            def emit_l1(ci):
                c0, f = chunks[ci]
                ps1 = psp.tile([128, 2 * F], FP, tag="ps", name=f"ps1_{ci}")
                off = m_off(f)
                for m in range(2):
                    nc.tensor.matmul(ps1[:, m * off:m * off + f],
                                     w_t[:, m * 128:(m + 1) * 128],
                                     emb_t[:, c0:c0 + f], start=True, stop=True)
                z1 = z1p.tile([128, 2 * F], mmdt, tag="z1", name=f"z1_{ci}")
                act_pair(z1, ps1, f, None if zero_bias else b1_t)
                z1s[ci] = z1

            def emit_l2(ci):
                c0, f = chunks[ci]
                z1 = z1s[ci]
                off = m_off(f)
                ps2 = psp.tile([128, 2 * F], FP, tag="ps", name=f"ps2_{ci}")
                for m in range(2):
                    nc.tensor.matmul(ps2[:, m * off:m * off + f],
                                     w_t[:, H + m * 128:H + m * 128 + 128],
                                     z1[:, :f], start=True, stop=False)
                    nc.tensor.matmul(ps2[:, m * off:m * off + f],
                                     w_t[:, 2 * H + m * 128:2 * H + m * 128 + 128],
                                     z1[:, off:off + f], start=False, stop=True)
                z2 = z2p.tile([128, 2 * F], mmdt, tag="z2", name=f"z2_{ci}")
                act_pair(z2, ps2, f, None if zero_bias else b2_t)
                z2s[ci], ps2s[ci] = z2, ps2

            def emit_l3(ci):
                c0, f = chunks[ci]
                z2 = z2s[ci]
                # L3 accumulates into a corner of ps2 after its ACT read
                # (WAR handled by Tile) -- no extra PSUM bank needed.
                ps3 = ps2s[ci][0:1, 0:f]
                off = m_off(f)
                nc.tensor.matmul(ps3, w_t[:, 3 * H:3 * H + 1], z2[:, :f],
                                 start=True, stop=False)
                nc.tensor.matmul(ps3, w_t[:, 3 * H + 1:3 * H + 2],
                                 z2[:, off:off + f], start=False, stop=True)
                if zero_bias:
                    nc.vector.tensor_copy(out_t[:, c0:c0 + f], ps3)
                else:
                    nc.vector.tensor_scalar_add(out_t[:, c0:c0 + f], ps3,
                                                b3_t[0:1, 0:1])

            depth = min(3, nch)
            for ci in range(depth):
                emit_l1(ci)
            for ci in range(nch):
                emit_l2(ci)
                if ci + depth < nch:
                    emit_l1(ci + depth)
                if ci >= 1:
                    emit_l3(ci - 1)
                    if ci == nch - 1:
                        # first span of the output ships while the last
                        # chunk finishes (out DMAs ride the idle sync ring)
                        bnd = chunks[ci - 1][0] + chunks[ci - 1][1]
                        nc.sync.dma_start(out_d[:, :bnd], out_t[:, :bnd])
            emit_l3(nch - 1)
            bnd = chunks[nch - 1][0] if nch > 1 else 0
            nc.sync.dma_start(out_d[:, bnd:npad], out_t[:, bnd:npad])

    nc.compile()
    return nc


def _route(species: np.ndarray):
    """species values -> expert idx (-1 unknown); per-core row assignments."""
    conv = np.full(MAXIDX + 2, -1, dtype=np.int32)
    conv[SPECIES_Z] = np.arange(NSPECIES, dtype=np.int32)
    idx = conv[species]
    core_rows = []
    for s in range(NSPECIES):
        rows = np.flatnonzero(idx == s)
        h = (len(rows) + 1) // 2
        core_rows.append(rows[:h])
        core_rows.append(rows[h:])
    return core_rows


def _run(inputs: dict, trace: bool = False, dtype_mode: str = "bf16"):
    species = inputs["species"]
    embedding = np.ascontiguousarray(inputs["embedding"], dtype=np.float32)
    n_atoms = species.shape[0]
    out_full = np.zeros((n_atoms, 1), dtype=np.float32)

    core_rows = _route(np.asarray(species))
    nmax = max(len(r) for r in core_rows)
    if nmax == 0:
        return out_full, None
    npad = -(-nmax // 8) * 8

    zero_bias = all(
        not np.any(np.asarray(inputs[k])) for k in ("b1", "b2", "b3"))
    mmdt = {"bf16": mybir.dt.bfloat16,
            "f32r": mybir.dt.float32r,
            "fp32": FP}[dtype_mode]
    np_mm = mybir.dt.np(mmdt)
    nc = _build_program(npad, zero_bias, mmdt)

    in_maps = []
    for c in range(N_CORES):
        s = c // 2
        rows = core_rows[c]
        embT = np.zeros((D, npad), dtype=np_mm)
        if len(rows):
            embT[:, :len(rows)] = embedding[rows].T.astype(np_mm)
        # combined weights tile: [w1 | w2 row-half 0 | w2 row-half 1 | w3]
        w = np.zeros((D, WCOLS), dtype=np_mm)
        w[:, 0:H] = np.asarray(inputs["W1"][s], dtype=np.float32).astype(np_mm)
        w2 = np.asarray(inputs["W2"][s], dtype=np.float32).astype(np_mm)
        w[:, H:2 * H] = w2[0:128, :]
        w[:, 2 * H:3 * H] = w2[128:256, :]
        w[:, 3 * H:3 * H + 2] = np.asarray(
            inputs["W3"][s], dtype=np.float32).reshape(2, 128).T.astype(np_mm)
        im = {
            "embT": embT,
            "w": np.ascontiguousarray(w),
        }
        if not zero_bias:
            im["b1"] = np.ascontiguousarray(
                np.asarray(inputs["b1"][s], dtype=np.float32).reshape(2, 128).T)
            im["b2"] = np.ascontiguousarray(
                np.asarray(inputs["b2"][s], dtype=np.float32).reshape(2, 128).T)
            im["b3"] = np.asarray(inputs["b3"][s], dtype=np.float32).reshape(1, 1)
        in_maps.append(im)

    res = run_bass_kernel_spmd(nc, in_maps, core_ids=list(range(N_CORES)),
                               trace=trace)
    for c in range(N_CORES):
        rows = core_rows[c]
        if len(rows):
            out_full[rows, 0] = res.results[c]["out"][0, :len(rows)]
    return out_full, res


def kernel(**inputs) -> np.ndarray:
    out, _ = _run(inputs, trace=False)
    return out
